# revision 5
# baseline (speedup 1.0000x reference)
"""Fused single-head cross-attention on 8 TRN2 NeuronCores (Bass/Tile).

Problem: out = (softmax(norm * (xWq+bq)(yWk+bk)^T + adj) @ (yWv+bv)) Wo + bo
Shapes: x,y [4, 2048, 1024], adj [4, 2048, 2048], all weights [1024, 1024].

Sharding: data-parallel over (batch, seq-half) -> 8 shards. Core c handles
batch b=c//2, query rows h*1024..(h+1)*1024 (h=c%2). K/V projections are
split across the core pair (each computes its own t-half of K^T and V) and
exchanged with pair-wise AllGathers.

v3 strategy (v2 was 310us, baseline 455us):
  * All matmul operands bf16 (same 1 col/cyc PE rate as f32r, half the
    DMA/SBUF, FWL-eligible weight loads). PSUM/denoms/output stay fp32.
  * Each AllGather is split in two halves launched mid-projection, so all
    four collectives (fixed ~10us engine latency each) complete long
    before attention consumes kT_all/v_all.
  * Per 1024-t slot: all 128 att matmuls first, then 128 AV matmuls that
    accumulate over the whole slot in PSUM (halves the DVE evacuations).
  * denacc accumulation on GpSimd (was DVE; DVE ADD is ~630ns/tile and
    was near-saturated); softmax reciprocal at [128,512] straight off
    partition_all_reduce output (the [1,512] reciprocal took 3.3us).
  * Phase O is sb-outer with resident Wo so its sb0 matmuls cover sb1's
    softmax finalize chain.
  * DMA discipline: input streams on the SP HWDGE queue in exact need
    order; compute-dependent stores on the ACT HWDGE queue; per-slot
    order [kp][adj][vp] so collective-gated loads never block earlier-
    needed data.
All attention math runs in "transposed" space:
    KT[d,t]   = matmul(lhsT=Wk, rhs=yT)                  (+bk per-partition)
    V [t,d]   = matmul(lhsT=yT, rhs=Wv)                  (+bv via gpsimd bcast)
    QT[d,s]   = matmul(lhsT=Wq, rhs=xT)                  (+bq per-partition)
    attT[t,s] = matmul(lhsT=KT, rhs=QT)  (+adjT via DVE, exp via ACT)
    numT[d,s] = matmul(lhsT=V,  rhs=exp)   (PSUM, accumulated per slot)
    denom[s]  = GpSimd-accumulated exp + gpsimd partition_all_reduce
    outT[d2,s]= matmul(lhsT=Wo, rhs=numT*recip(denom))   (+bo per-partition)
  softmax max-subtraction is skipped: logits are O(1) by construction.
"""
import sys

if "/opt/trn_rl_repo" not in sys.path:
    sys.path.insert(0, "/opt/trn_rl_repo")

import numpy as np
import ml_dtypes

import concourse.bass as bass
import concourse.bass_isa as bass_isa
import concourse.tile as tile
from concourse import bacc, mybir
from concourse.bass_utils import run_bass_kernel_spmd

P = 128
D = 1024
S = 2048
SC = 1024            # per-core query rows
TH = 1024            # per-core own K/V t-half
DC = D // P          # 8 feature chunks
SB = 512             # matmul moving free dim
NSB = SC // SB       # 2 s blocks
TTP = 4              # t-tiles (128) per 512-panel
NORM = 1.0 / 32.0
GROUPS = [[0, 1], [2, 3], [4, 5], [6, 7]]

F32 = mybir.dt.float32
BF16 = mybir.dt.bfloat16
ID = mybir.ActivationFunctionType.Identity
EXP = mybir.ActivationFunctionType.Exp
BF16NP = ml_dtypes.bfloat16

_CACHE = {}


def _mm(nc, ps, lhsT, rhs, start, stop):
    nc.tensor.matmul(ps, lhsT=lhsT, rhs=rhs, start=start, stop=stop)


def build_nc():
    nc = bacc.Bacc("TRN2", target_bir_lowering=False, debug=False, num_devices=8)

    xT = nc.dram_tensor("xT", [D, SC], BF16, kind="ExternalInput")
    yT = nc.dram_tensor("yT", [D, TH], BF16, kind="ExternalInput")  # own t-half
    adjT = nc.dram_tensor("adjT", [S, SC], BF16, kind="ExternalInput")
    # weights pre-tiled on host: Wx_t[dt][p][c][col] = Wx[c*P+p, dt*P+col]
    Wq = nc.dram_tensor("Wq", [DC, P, DC, P], BF16, kind="ExternalInput")
    Wk = nc.dram_tensor("Wk", [DC, P, DC, P], BF16, kind="ExternalInput")
    Wo = nc.dram_tensor("Wo", [DC, P, DC, P], BF16, kind="ExternalInput")
    # Wv pre-tiled as rhs: Wv_t[db][p][c][col] = Wv[c*P+p, db*SB+col]
    Wv = nc.dram_tensor("Wv", [2, P, DC, SB], BF16, kind="ExternalInput")
    bq = nc.dram_tensor("bq", [P, DC], F32, kind="ExternalInput")
    bk = nc.dram_tensor("bk", [P, DC], F32, kind="ExternalInput")
    bv = nc.dram_tensor("bv", [1, D], F32, kind="ExternalInput")
    bo = nc.dram_tensor("bo", [P, DC], F32, kind="ExternalInput")
    outT = nc.dram_tensor("outT", [D, SC], F32, kind="ExternalOutput")

    # pair exchange (bf16), each tensor split in two for earlier collectives
    kT_loc = [nc.dram_tensor(f"kT_loc{i}", [D, SB], BF16) for i in range(2)]
    v_loc = [nc.dram_tensor(f"v_loc{i}", [SB, D], BF16) for i in range(2)]
    kT_all = [nc.dram_tensor(f"kT_all{i}", [2, D, SB], BF16) for i in range(2)]
    v_all = [nc.dram_tensor(f"v_all{i}", [2, SB, D], BF16) for i in range(2)]

    xT_r = xT.rearrange("(c p) s -> p c s", p=P)
    yT_r = yT.rearrange("(c p) t -> p c t", p=P)
    kT_all_r = [t.rearrange("r (c p) t -> r p c t", p=P) for t in kT_all]
    v_all_r = [t.rearrange("r (j p) d -> r p j d", p=P) for t in v_all]

    with tile.TileContext(nc) as tc:
        with (
            nc.allow_low_precision(reason="bf16 operands keep rel err ~2e-3"),
            tc.tile_pool(name="res", bufs=1) as res,
        ):
            # ---- resident tiles --------------------------------------
            QT_sb = res.tile([P, DC, SC], BF16, name="QT_sb")
            num_sb = res.tile([P, DC, SC], F32, name="num_sb")
            scaled = res.tile([P, NSB, DC, SB], BF16, name="scaled")
            denacc = res.tile([P, NSB, SB], F32, name="denacc")
            dsum = res.tile([P, SB], F32, name="dsum")
            rb = res.tile([P, NSB, SB], F32, name="rb")
            bv_bc = res.tile([P, D], F32, name="bv_bc")
            bq_sb = res.tile([P, DC], F32, name="bq_sb")
            bk_sb = res.tile([P, DC], F32, name="bk_sb")
            bo_sb = res.tile([P, DC], F32, name="bo_sb")
            bv_sb = res.tile([1, D], F32, name="bv_sb")
            nc.sync.dma_start(out=bk_sb[:], in_=bk[:])
            nc.sync.dma_start(out=bv_sb[:], in_=bv[:])
            nc.sync.dma_start(out=bq_sb[:], in_=bq[:])
            nc.sync.dma_start(out=bo_sb[:], in_=bo[:])
            nc.gpsimd.partition_broadcast(bv_bc[:], bv_sb[0:1, :], channels=P)

            with (
                tc.tile_pool(name="qkv_in", bufs=1) as qkvp,
                tc.tile_pool(name="wk_pool", bufs=1) as wkp,
                tc.tile_pool(name="wq_pool", bufs=4) as wqp,
                tc.tile_pool(name="wv_pool", bufs=1) as wvp,
                tc.tile_pool(name="kv_out", bufs=3) as kvo,
                tc.tile_pool(name="qkv_ps", bufs=3, space="PSUM") as qps,
            ):
                yT_sb = qkvp.tile([P, DC, TH], BF16, name="yT_sb")
                xT_sb = qkvp.tile([P, DC, SC], BF16, name="xT_sb")
                wv_t = [wvp.tile([P, DC, SB], BF16, name=f"wv{i}") for i in range(2)]
                wk_t = [wkp.tile([P, DC, P], BF16, name=f"wk{i}") for i in range(DC)]

                # ---- phase K: KT(own half) = Wk^T y^T + bk -----------
                # first-needed loads first: wk0, then yT full rows
                nc.sync.dma_start(out=wk_t[0][:], in_=Wk[0])
                for c in range(DC):
                    nc.sync.dma_start(out=yT_sb[:, c, :], in_=yT_r[:, c, :])
                for dt in range(1, DC):
                    nc.sync.dma_start(out=wk_t[dt][:], in_=Wk[dt])
                # tb-outer so each half's exchange can launch mid-phase
                for tb in range(NSB):
                    for dt in range(DC):
                        ps = qps.tile([P, SB], F32, name="k_ps", tag="qkvps")
                        for c in range(DC):
                            _mm(
                                nc, ps[:],
                                wk_t[dt][:, c, :],
                                yT_sb[:, c, tb * SB : (tb + 1) * SB],
                                c == 0, c == DC - 1,
                            )
                        kt = kvo.tile([P, SB], BF16, name="kt")
                        nc.scalar.activation(
                            out=kt[:], in_=ps[:], func=ID,
                            bias=bk_sb[:, dt : dt + 1],
                        )
                        nc.scalar.dma_start(
                            out=kT_loc[tb][dt * P : (dt + 1) * P, :], in_=kt[:]
                        )
                    nc.gpsimd.collective_compute(
                        "AllGather", mybir.AluOpType.bypass,
                        replica_groups=GROUPS,
                        ins=[kT_loc[tb][:]], outs=[kT_all[tb][:]],
                    )

                # wv + xT input streams (consumed by phases V and Q)
                for db in range(2):
                    nc.sync.dma_start(out=wv_t[db][:], in_=Wv[db])
                for c in range(DC):
                    nc.sync.dma_start(out=xT_sb[:, c, :], in_=xT_r[:, c, :])

                # ---- phase V: V(own half) = y Wv + bv ----------------
                for th in range(2):
                    for tl in range(SB // P):
                        tt = th * (SB // P) + tl
                        for db in range(2):
                            ps = qps.tile([P, SB], F32, name="v_ps", tag="qkvps")
                            for c in range(DC):
                                _mm(
                                    nc, ps[:],
                                    yT_sb[:, c, tt * P : (tt + 1) * P],
                                    wv_t[db][:, c, :],
                                    c == 0, c == DC - 1,
                                )
                            vt = kvo.tile([P, SB], BF16, name="vt")
                            nc.vector.tensor_add(
                                vt[:], ps[:], bv_bc[:, db * SB : (db + 1) * SB]
                            )
                            nc.scalar.dma_start(
                                out=v_loc[th][tl * P : (tl + 1) * P,
                                              db * SB : (db + 1) * SB],
                                in_=vt[:],
                            )
                    nc.gpsimd.collective_compute(
                        "AllGather", mybir.AluOpType.bypass,
                        replica_groups=GROUPS,
                        ins=[v_loc[th][:]], outs=[v_all[th][:]],
                    )

                # ---- phase Q: QT = Wq^T x^T + bq ---------------------
                for dt in range(DC):
                    wq = wqp.tile([P, DC, P], BF16, name="wq_t", tag="w")
                    nc.sync.dma_start(out=wq[:], in_=Wq[dt])
                    for sb in range(NSB):
                        ps = qps.tile([P, SB], F32, name="q_ps", tag="qkvps")
                        for c in range(DC):
                            _mm(
                                nc, ps[:],
                                wq[:, c, :],
                                xT_sb[:, c, sb * SB : (sb + 1) * SB],
                                c == 0, c == DC - 1,
                            )
                        nc.scalar.activation(
                            out=QT_sb[:, dt, sb * SB : (sb + 1) * SB],
                            in_=ps[:], func=ID, bias=bq_sb[:, dt : dt + 1],
                        )

            # ---- phase A: attention, slot (pair member) outer --------
            with (
                tc.tile_pool(name="kp_pool", bufs=2) as kpp,
                tc.tile_pool(name="vp_pool", bufs=2) as vpp,
                tc.tile_pool(name="exp_pool", bufs=5) as expp,
                tc.tile_pool(name="adj_pool", bufs=18) as adjp,
                tc.tile_pool(name="tmp_pool", bufs=3) as tmpp,
                tc.tile_pool(name="aps", bufs=3, space="PSUM") as aps,
                tc.tile_pool(name="nps", bufs=5, space="PSUM") as npsp,
            ):
                for r in range(2):
                    # collective-gated kp first, then this slot's adj
                    # stream, then collective-gated vp (needed last)
                    kp = kpp.tile([P, DC, TH], BF16, name="kp")
                    for lb in range(2):
                        for c in range(DC):
                            nc.sync.dma_start(
                                out=kp[:, c, lb * SB : (lb + 1) * SB],
                                in_=kT_all_r[lb][r, :, c, :],
                            )
                    ats = {}
                    for lb in range(2):
                        for sb in range(NSB):
                            ssl = slice(sb * SB, (sb + 1) * SB)
                            for tt in range(TTP):
                                tg = (r * 2 + lb) * TTP + tt
                                at = adjp.tile([P, SB], BF16, name="at")
                                nc.sync.dma_start(
                                    out=at[:],
                                    in_=adjT[tg * P : (tg + 1) * P, ssl],
                                )
                                ats[(lb, sb, tt)] = at
                    vp = vpp.tile([P, TH // P, D], BF16, name="vp")
                    for j in range(TH // P):
                        nc.sync.dma_start(
                            out=vp[:, j, :], in_=v_all_r[j // TTP][r, :, j % TTP, :]
                        )

                    # att for the whole slot (128 mm), exp as we go
                    exs = {}
                    for lb in range(2):
                        for sb in range(NSB):
                            ssl = slice(sb * SB, (sb + 1) * SB)
                            ex = expp.tile([P, TTP, SB], BF16, name="ex")
                            exs[(lb, sb)] = ex
                            for tt in range(TTP):
                                att = aps.tile([P, SB], F32, name="att")
                                for c in range(DC):
                                    _mm(
                                        nc, att[:],
                                        kp[:, c, lb * SB + tt * P
                                           : lb * SB + (tt + 1) * P],
                                        QT_sb[:, c, ssl],
                                        c == 0, c == DC - 1,
                                    )
                                tm = tmpp.tile([P, SB], F32, name="tm")
                                nc.vector.tensor_add(
                                    tm[:], att[:], ats[(lb, sb, tt)][:]
                                )
                                nc.scalar.activation(
                                    out=ex[:, tt, :], in_=tm[:], func=EXP
                                )
                                if r == 0 and lb == 0 and tt == 0:
                                    nc.gpsimd.tensor_copy(
                                        denacc[:, sb, :], ex[:, tt, :]
                                    )
                                else:
                                    nc.gpsimd.tensor_add(
                                        denacc[:, sb, :], denacc[:, sb, :],
                                        ex[:, tt, :],
                                    )
                    # AV for the whole slot: PSUM-accumulate all 8 t-tiles
                    for sb in range(NSB):
                        ssl = slice(sb * SB, (sb + 1) * SB)
                        for dh in range(2):
                            nt = [
                                npsp.tile([P, SB], F32, name="np")
                                for _ in range(DC // 2)
                            ]
                            for lb in range(2):
                                ex = exs[(lb, sb)]
                                for tt in range(TTP):
                                    for d4 in range(DC // 2):
                                        _mm(
                                            nc, nt[d4][:],
                                            vp[:, lb * TTP + tt,
                                               (dh * 4 + d4) * P
                                               : (dh * 4 + d4 + 1) * P],
                                            ex[:, tt, :],
                                            lb == 0 and tt == 0,
                                            lb == 1 and tt == TTP - 1,
                                        )
                            for d4 in range(DC // 2):
                                dst = num_sb[:, dh * 4 + d4, ssl]
                                if r == 0:
                                    nc.vector.tensor_copy(dst, nt[d4][:])
                                else:
                                    nc.vector.tensor_add(dst, dst, nt[d4][:])
                        if r == 1:
                            # finalize softmax scale for this s-block while
                            # the other s-block still computes; the
                            # all-reduce leaves the sum on every partition
                            nc.gpsimd.partition_all_reduce(
                                dsum[:], denacc[:, sb, :],
                                channels=P, reduce_op=bass_isa.ReduceOp.add,
                            )
                            nc.vector.reciprocal(rb[:, sb, :], dsum[:])
                            for c in range(DC):
                                nc.vector.tensor_mul(
                                    scaled[:, sb, c, :],
                                    num_sb[:, c, ssl],
                                    rb[:, sb, :],
                                )

            # ---- phase O: out^T = Wo^T (numT*recip) + bo -------------
            with (
                tc.tile_pool(name="wo_pool", bufs=1) as wop,
                tc.tile_pool(name="o_out", bufs=3) as oout,
                tc.tile_pool(name="ops", bufs=3, space="PSUM") as ops,
            ):
                wo_t = [wop.tile([P, DC, P], BF16, name=f"wo{i}") for i in range(DC)]
                for dt in range(DC):
                    nc.sync.dma_start(out=wo_t[dt][:], in_=Wo[dt])
                # sb-outer: sb0 matmuls run while sb1's finalize completes
                for sb in range(NSB):
                    for dt in range(DC):
                        po = ops.tile([P, SB], F32, name="po")
                        for c in range(DC):
                            _mm(
                                nc, po[:],
                                wo_t[dt][:, c, :],
                                scaled[:, sb, c, :],
                                c == 0, c == DC - 1,
                            )
                        ot = oout.tile([P, SB], F32, name="ot")
                        nc.scalar.activation(
                            out=ot[:], in_=po[:], func=ID,
                            bias=bo_sb[:, dt : dt + 1],
                        )
                        nc.scalar.dma_start(
                            out=outT[dt * P : (dt + 1) * P,
                                     sb * SB : (sb + 1) * SB],
                            in_=ot[:],
                        )
    nc.compile()
    return nc


def _get_nc():
    if "nc" not in _CACHE:
        _CACHE["nc"] = build_nc()
    return _CACHE["nc"]


def _tile_lhs(W):
    # [dt][p][c][col] = W[c*P+p, dt*P+col]
    return np.ascontiguousarray(
        W.reshape(DC, P, DC, P).transpose(2, 1, 0, 3).astype(BF16NP)
    )


def kernel(x, y, adj, Wq, bq, Wk, bk, Wv, bv, Wo, bo, _trace=False):
    x = np.asarray(x, dtype=np.float32)
    y = np.asarray(y, dtype=np.float32)
    adj = np.asarray(adj, dtype=np.float32)
    Wq_h = _tile_lhs(np.asarray(Wq, np.float32) * NORM)
    Wk_h = _tile_lhs(np.asarray(Wk, np.float32))
    Wo_h = _tile_lhs(np.asarray(Wo, np.float32))
    # Wv as rhs tiles: [db][p][c][col] = Wv[c*P+p, db*SB+col]
    Wv_h = np.ascontiguousarray(
        np.asarray(Wv, np.float32).reshape(DC, P, 2, SB)
        .transpose(2, 1, 0, 3).astype(BF16NP)
    )
    bq_s = np.asarray(bq, np.float32) * NORM
    bq_h = np.ascontiguousarray(bq_s.reshape(DC, P).T)
    bk_h = np.ascontiguousarray(np.asarray(bk, np.float32).reshape(DC, P).T)
    bo_h = np.ascontiguousarray(np.asarray(bo, np.float32).reshape(DC, P).T)
    bv_h = np.ascontiguousarray(np.asarray(bv, np.float32).reshape(1, D))

    in_maps = []
    for c in range(8):
        b, h = c // 2, c % 2
        ssl = slice(h * SC, (h + 1) * SC)
        in_maps.append(
            {
                "xT": np.ascontiguousarray(x[b, ssl, :].T.astype(BF16NP)),
                "yT": np.ascontiguousarray(y[b, ssl, :].T.astype(BF16NP)),
                "adjT": np.ascontiguousarray(adj[b, ssl, :].T.astype(BF16NP)),
                "Wq": Wq_h, "Wk": Wk_h, "Wv": Wv_h, "Wo": Wo_h,
                "bq": bq_h, "bk": bk_h, "bv": bv_h, "bo": bo_h,
            }
        )

    nc = _get_nc()
    res = run_bass_kernel_spmd(nc, in_maps, list(range(8)), trace=_trace)
    if _trace:
        _CACHE["last_exec_time_ns"] = res.exec_time_ns
        _CACHE["last_trace"] = (
            res.instructions_and_trace[1] if res.instructions_and_trace else None
        )

    out = np.empty((4, S, D), np.float32)
    for c in range(8):
        b, h = c // 2, c % 2
        out[b, h * SC : (h + 1) * SC, :] = res.results[c]["outT"].T
    return out


# revision 6
# speedup vs baseline: 1.0769x; 1.0769x over previous
"""Fused single-head cross-attention on 8 TRN2 NeuronCores (Bass/Tile).

Problem: out = (softmax(norm * (xWq+bq)(yWk+bk)^T + adj) @ (yWv+bv)) Wo + bo
Shapes: x,y [4, 2048, 1024], adj [4, 2048, 2048], all weights [1024, 1024].

Sharding: data-parallel over (batch, seq-half) -> 8 shards. Core c handles
batch b=c//2, query rows h*1024..(h+1)*1024 (h=c%2). K/V projections are
split across the core pair (each computes its own t-half of K^T and V) and
exchanged with one pair-wise AllGather each.

v4 strategy (v2 310us, v3 318us, baseline 455us):
  * All matmul operands bf16 (same 1 col/cyc PE rate as f32r, half the
    DMA/SBUF, FWL-eligible weight loads). PSUM/denoms/output stay fp32.
  * Two AllGathers only (splitting them 4-way serialized ~20us each).
    The K exchange launches mid-V, the V exchange mid-Q; both complete
    well before attention consumes them (attention now reads V only
    after the full slot's QK+exp work, 28us later than v2's schedule).
  * Per 1024-t slot: all 128 att matmuls first, then 128 AV matmuls that
    accumulate over the whole slot in PSUM (halves the DVE evacuations).
  * wq streams on the ACT HWDGE queue, issued after the kt stores, so the
    store descriptors are not queued behind 4MB of weight loads on the
    shared HW rings (cost v3 a 12us Q stall).
  * Phase O runs inside the attention pool scope and reuses its PSUM
    pool, so there is no pool-teardown barrier before the first out
    matmul (cost v3 26us).
  * denacc accumulation on GpSimd for slot 0, DVE for slot 1; softmax
    reciprocal at [128,512] straight off partition_all_reduce output.
All attention math runs in "transposed" space:
    KT[d,t]   = matmul(lhsT=Wk, rhs=yT)                  (+bk per-partition)
    V [t,d]   = matmul(lhsT=yT, rhs=Wv)                  (+bv via gpsimd bcast)
    QT[d,s]   = matmul(lhsT=Wq, rhs=xT)                  (+bq per-partition)
    attT[t,s] = matmul(lhsT=KT, rhs=QT)  (+adjT via DVE, exp via ACT)
    numT[d,s] = matmul(lhsT=V,  rhs=exp)   (PSUM, accumulated per slot)
    denom[s]  = accumulated exp + gpsimd partition_all_reduce
    outT[d2,s]= matmul(lhsT=Wo, rhs=numT*recip(denom))   (+bo per-partition)
  softmax max-subtraction is skipped: logits are O(1) by construction.
"""
import sys

if "/opt/trn_rl_repo" not in sys.path:
    sys.path.insert(0, "/opt/trn_rl_repo")

import numpy as np
import ml_dtypes

import concourse.bass as bass
import concourse.bass_isa as bass_isa
import concourse.tile as tile
from concourse import bacc, mybir
from concourse.bass_utils import run_bass_kernel_spmd

P = 128
D = 1024
S = 2048
SC = 1024            # per-core query rows
TH = 1024            # per-core own K/V t-half
DC = D // P          # 8 feature chunks
SB = 512             # matmul moving free dim
NSB = SC // SB       # 2 s blocks
TTP = 4              # t-tiles (128) per 512-panel
NORM = 1.0 / 32.0
GROUPS = [[0, 1], [2, 3], [4, 5], [6, 7]]

F32 = mybir.dt.float32
BF16 = mybir.dt.bfloat16
ID = mybir.ActivationFunctionType.Identity
EXP = mybir.ActivationFunctionType.Exp
BF16NP = ml_dtypes.bfloat16

_CACHE = {}


def _mm(nc, ps, lhsT, rhs, start, stop):
    nc.tensor.matmul(ps, lhsT=lhsT, rhs=rhs, start=start, stop=stop)


def build_nc():
    nc = bacc.Bacc("TRN2", target_bir_lowering=False, debug=False, num_devices=8)

    xT = nc.dram_tensor("xT", [D, SC], BF16, kind="ExternalInput")
    yT = nc.dram_tensor("yT", [D, TH], BF16, kind="ExternalInput")  # own t-half
    adjT = nc.dram_tensor("adjT", [S, SC], BF16, kind="ExternalInput")
    # weights pre-tiled on host: Wx_t[dt][p][c][col] = Wx[c*P+p, dt*P+col]
    Wq = nc.dram_tensor("Wq", [DC, P, DC, P], BF16, kind="ExternalInput")
    Wk = nc.dram_tensor("Wk", [DC, P, DC, P], BF16, kind="ExternalInput")
    Wo = nc.dram_tensor("Wo", [DC, P, DC, P], BF16, kind="ExternalInput")
    # Wv pre-tiled as rhs: Wv_t[db][p][c][col] = Wv[c*P+p, db*SB+col]
    Wv = nc.dram_tensor("Wv", [2, P, DC, SB], BF16, kind="ExternalInput")
    bq = nc.dram_tensor("bq", [P, DC], F32, kind="ExternalInput")
    bk = nc.dram_tensor("bk", [P, DC], F32, kind="ExternalInput")
    bv = nc.dram_tensor("bv", [1, D], F32, kind="ExternalInput")
    bo = nc.dram_tensor("bo", [P, DC], F32, kind="ExternalInput")
    outT = nc.dram_tensor("outT", [D, SC], F32, kind="ExternalOutput")

    # pair exchange tensors (bf16)
    kT_loc = nc.dram_tensor("kT_loc", [D, TH], BF16)
    v_loc = nc.dram_tensor("v_loc", [TH, D], BF16)
    kT_all = nc.dram_tensor("kT_all", [2, D, TH], BF16)
    v_all = nc.dram_tensor("v_all", [2, TH, D], BF16)

    xT_r = xT.rearrange("(c p) s -> p c s", p=P)
    yT_r = yT.rearrange("(c p) t -> p c t", p=P)
    kT_all_r = kT_all.rearrange("r (c p) t -> r p c t", p=P)
    v_all_r = v_all.rearrange("r (j p) d -> r p j d", p=P)

    with tile.TileContext(nc) as tc:
        with (
            nc.allow_low_precision(reason="bf16 operands keep rel err ~2e-3"),
            tc.tile_pool(name="res", bufs=1) as res,
        ):
            # ---- resident tiles --------------------------------------
            QT_sb = res.tile([P, DC, SC], BF16, name="QT_sb")
            num_sb = res.tile([P, DC, SC], F32, name="num_sb")
            scaled = res.tile([P, NSB, DC, SB], BF16, name="scaled")
            denacc = res.tile([P, NSB, SB], F32, name="denacc")
            dsum = res.tile([P, SB], F32, name="dsum")
            rb = res.tile([P, NSB, SB], F32, name="rb")
            bv_bc = res.tile([P, D], F32, name="bv_bc")
            bq_sb = res.tile([P, DC], F32, name="bq_sb")
            bk_sb = res.tile([P, DC], F32, name="bk_sb")
            bo_sb = res.tile([P, DC], F32, name="bo_sb")
            bv_sb = res.tile([1, D], F32, name="bv_sb")
            nc.sync.dma_start(out=bk_sb[:], in_=bk[:])
            nc.sync.dma_start(out=bv_sb[:], in_=bv[:])
            nc.sync.dma_start(out=bq_sb[:], in_=bq[:])
            nc.sync.dma_start(out=bo_sb[:], in_=bo[:])
            nc.gpsimd.partition_broadcast(bv_bc[:], bv_sb[0:1, :], channels=P)

            with (
                tc.tile_pool(name="qkv_in", bufs=1) as qkvp,
                tc.tile_pool(name="wk_pool", bufs=1) as wkp,
                tc.tile_pool(name="wq_pool", bufs=1) as wqp,
                tc.tile_pool(name="wv_pool", bufs=1) as wvp,
                tc.tile_pool(name="kv_out", bufs=3) as kvo,
                tc.tile_pool(name="qkv_ps", bufs=3, space="PSUM") as qps,
            ):
                yT_sb = qkvp.tile([P, DC, TH], BF16, name="yT_sb")
                xT_sb = qkvp.tile([P, DC, SC], BF16, name="xT_sb")
                wv_t = [wvp.tile([P, DC, SB], BF16, name=f"wv{i}") for i in range(2)]
                wk_t = [wkp.tile([P, DC, P], BF16, name=f"wk{i}") for i in range(DC)]
                wq_t = [wqp.tile([P, DC, P], BF16, name=f"wq{i}") for i in range(DC)]

                # ---- phase K: KT(own half) = Wk^T y^T + bk -----------
                # first-needed loads first: wk0, then yT full rows
                nc.sync.dma_start(out=wk_t[0][:], in_=Wk[0])
                for c in range(DC):
                    nc.sync.dma_start(out=yT_sb[:, c, :], in_=yT_r[:, c, :])
                for dt in range(1, DC):
                    nc.sync.dma_start(out=wk_t[dt][:], in_=Wk[dt])
                for tb in range(NSB):
                    for dt in range(DC):
                        ps = qps.tile([P, SB], F32, name="k_ps", tag="qkvps")
                        for c in range(DC):
                            _mm(
                                nc, ps[:],
                                wk_t[dt][:, c, :],
                                yT_sb[:, c, tb * SB : (tb + 1) * SB],
                                c == 0, c == DC - 1,
                            )
                        kt = kvo.tile([P, SB], BF16, name="kt")
                        nc.scalar.activation(
                            out=kt[:], in_=ps[:], func=ID,
                            bias=bk_sb[:, dt : dt + 1],
                        )
                        nc.scalar.dma_start(
                            out=kT_loc[dt * P : (dt + 1) * P,
                                       tb * SB : (tb + 1) * SB],
                            in_=kt[:],
                        )
                nc.gpsimd.collective_compute(
                    "AllGather", mybir.AluOpType.bypass,
                    replica_groups=GROUPS,
                    ins=[kT_loc[:]], outs=[kT_all[:]],
                )
                # wq on the ACT queue: issues after the kt stores, so the
                # store descriptors win the HW rings during phase K
                for dt in range(DC):
                    nc.scalar.dma_start(out=wq_t[dt][:], in_=Wq[dt])

                # wv + xT input streams (consumed by phases V and Q)
                for db in range(2):
                    nc.sync.dma_start(out=wv_t[db][:], in_=Wv[db])
                for c in range(DC):
                    nc.sync.dma_start(out=xT_sb[:, c, :], in_=xT_r[:, c, :])

                # ---- phase V: V(own half) = y Wv + bv ----------------
                for tt in range(TH // P):
                    for db in range(2):
                        ps = qps.tile([P, SB], F32, name="v_ps", tag="qkvps")
                        for c in range(DC):
                            _mm(
                                nc, ps[:],
                                yT_sb[:, c, tt * P : (tt + 1) * P],
                                wv_t[db][:, c, :],
                                c == 0, c == DC - 1,
                            )
                        vt = kvo.tile([P, SB], BF16, name="vt")
                        nc.vector.tensor_add(
                            vt[:], ps[:], bv_bc[:, db * SB : (db + 1) * SB]
                        )
                        nc.scalar.dma_start(
                            out=v_loc[tt * P : (tt + 1) * P,
                                      db * SB : (db + 1) * SB],
                            in_=vt[:],
                        )
                nc.gpsimd.collective_compute(
                    "AllGather", mybir.AluOpType.bypass,
                    replica_groups=GROUPS,
                    ins=[v_loc[:]], outs=[v_all[:]],
                )

                # ---- phase Q: QT = Wq^T x^T + bq ---------------------
                for dt in range(DC):
                    for sb in range(NSB):
                        ps = qps.tile([P, SB], F32, name="q_ps", tag="qkvps")
                        for c in range(DC):
                            _mm(
                                nc, ps[:],
                                wq_t[dt][:, c, :],
                                xT_sb[:, c, sb * SB : (sb + 1) * SB],
                                c == 0, c == DC - 1,
                            )
                        nc.scalar.activation(
                            out=QT_sb[:, dt, sb * SB : (sb + 1) * SB],
                            in_=ps[:], func=ID, bias=bq_sb[:, dt : dt + 1],
                        )

            # ---- phase A + O share pools (no teardown barrier) -------
            with (
                tc.tile_pool(name="kp_pool", bufs=2) as kpp,
                tc.tile_pool(name="vp_pool", bufs=2) as vpp,
                tc.tile_pool(name="exp_pool", bufs=5) as expp,
                tc.tile_pool(name="adj_pool", bufs=18) as adjp,
                tc.tile_pool(name="tmp_pool", bufs=3) as tmpp,
                tc.tile_pool(name="wo_pool", bufs=1) as wop,
                tc.tile_pool(name="aps", bufs=3, space="PSUM") as aps,
                tc.tile_pool(name="nps", bufs=5, space="PSUM") as npsp,
            ):
                for r in range(2):
                    # collective-gated kp first, then this slot's adj
                    # stream, then collective-gated vp (needed last)
                    kp = kpp.tile([P, DC, TH], BF16, name="kp")
                    for c in range(DC):
                        nc.sync.dma_start(out=kp[:, c, :], in_=kT_all_r[r, :, c, :])
                    ats = {}
                    for lb in range(2):
                        for sb in range(NSB):
                            ssl = slice(sb * SB, (sb + 1) * SB)
                            for tt in range(TTP):
                                tg = (r * 2 + lb) * TTP + tt
                                at = adjp.tile([P, SB], BF16, name="at")
                                nc.sync.dma_start(
                                    out=at[:],
                                    in_=adjT[tg * P : (tg + 1) * P, ssl],
                                )
                                ats[(lb, sb, tt)] = at
                    vp = vpp.tile([P, TH // P, D], BF16, name="vp")
                    for j in range(TH // P):
                        nc.sync.dma_start(out=vp[:, j, :], in_=v_all_r[r, :, j, :])
                    if r == 1:
                        # Wo stream for phase O (gated behind vp's
                        # collective wait; lands ~100us before use)
                        for dt in range(DC):
                            nc.sync.dma_start(out=wo_t[dt][:], in_=Wo[dt])

                    if r == 0:
                        wo_t = [
                            wop.tile([P, DC, P], BF16, name=f"wo{i}")
                            for i in range(DC)
                        ]

                    # att for the whole slot (128 mm), exp as we go
                    exs = {}
                    for lb in range(2):
                        for sb in range(NSB):
                            ssl = slice(sb * SB, (sb + 1) * SB)
                            ex = expp.tile([P, TTP, SB], BF16, name="ex")
                            exs[(lb, sb)] = ex
                            for tt in range(TTP):
                                att = aps.tile([P, SB], F32, name="att")
                                for c in range(DC):
                                    _mm(
                                        nc, att[:],
                                        kp[:, c, lb * SB + tt * P
                                           : lb * SB + (tt + 1) * P],
                                        QT_sb[:, c, ssl],
                                        c == 0, c == DC - 1,
                                    )
                                tm = tmpp.tile([P, SB], F32, name="tm")
                                nc.vector.tensor_add(
                                    tm[:], att[:], ats[(lb, sb, tt)][:]
                                )
                                nc.scalar.activation(
                                    out=ex[:, tt, :], in_=tm[:], func=EXP
                                )
                                eng = nc.gpsimd if r == 0 else nc.vector
                                if r == 0 and lb == 0 and tt == 0:
                                    eng.tensor_copy(
                                        denacc[:, sb, :], ex[:, tt, :]
                                    )
                                else:
                                    eng.tensor_add(
                                        denacc[:, sb, :], denacc[:, sb, :],
                                        ex[:, tt, :],
                                    )
                    # AV for the whole slot: PSUM-accumulate all 8 t-tiles
                    for sb in range(NSB):
                        ssl = slice(sb * SB, (sb + 1) * SB)
                        for dh in range(2):
                            nt = [
                                npsp.tile([P, SB], F32, name="np")
                                for _ in range(DC // 2)
                            ]
                            for lb in range(2):
                                ex = exs[(lb, sb)]
                                for tt in range(TTP):
                                    for d4 in range(DC // 2):
                                        _mm(
                                            nc, nt[d4][:],
                                            vp[:, lb * TTP + tt,
                                               (dh * 4 + d4) * P
                                               : (dh * 4 + d4 + 1) * P],
                                            ex[:, tt, :],
                                            lb == 0 and tt == 0,
                                            lb == 1 and tt == TTP - 1,
                                        )
                            for d4 in range(DC // 2):
                                dst = num_sb[:, dh * 4 + d4, ssl]
                                if r == 0:
                                    nc.vector.tensor_copy(dst, nt[d4][:])
                                else:
                                    nc.vector.tensor_add(dst, dst, nt[d4][:])
                        if r == 1:
                            # finalize softmax scale for this s-block while
                            # the other s-block still computes; the
                            # all-reduce leaves the sum on every partition
                            nc.gpsimd.partition_all_reduce(
                                dsum[:], denacc[:, sb, :],
                                channels=P, reduce_op=bass_isa.ReduceOp.add,
                            )
                            nc.vector.reciprocal(rb[:, sb, :], dsum[:])
                            for c in range(DC):
                                nc.vector.tensor_mul(
                                    scaled[:, sb, c, :],
                                    num_sb[:, c, ssl],
                                    rb[:, sb, :],
                                )

                # ---- phase O: out^T = Wo^T (numT*recip) + bo ---------
                # sb-outer: sb0 matmuls run while sb1's finalize completes
                for sb in range(NSB):
                    for dt in range(DC):
                        po = aps.tile([P, SB], F32, name="att")
                        for c in range(DC):
                            _mm(
                                nc, po[:],
                                wo_t[dt][:, c, :],
                                scaled[:, sb, c, :],
                                c == 0, c == DC - 1,
                            )
                        ot = tmpp.tile([P, SB], F32, name="tm")
                        nc.scalar.activation(
                            out=ot[:], in_=po[:], func=ID,
                            bias=bo_sb[:, dt : dt + 1],
                        )
                        nc.scalar.dma_start(
                            out=outT[dt * P : (dt + 1) * P,
                                     sb * SB : (sb + 1) * SB],
                            in_=ot[:],
                        )
    nc.compile()
    return nc


def _get_nc():
    if "nc" not in _CACHE:
        _CACHE["nc"] = build_nc()
    return _CACHE["nc"]


def _tile_lhs(W):
    # [dt][p][c][col] = W[c*P+p, dt*P+col]
    return np.ascontiguousarray(
        W.reshape(DC, P, DC, P).transpose(2, 1, 0, 3).astype(BF16NP)
    )


def kernel(x, y, adj, Wq, bq, Wk, bk, Wv, bv, Wo, bo, _trace=False):
    x = np.asarray(x, dtype=np.float32)
    y = np.asarray(y, dtype=np.float32)
    adj = np.asarray(adj, dtype=np.float32)
    Wq_h = _tile_lhs(np.asarray(Wq, np.float32) * NORM)
    Wk_h = _tile_lhs(np.asarray(Wk, np.float32))
    Wo_h = _tile_lhs(np.asarray(Wo, np.float32))
    # Wv as rhs tiles: [db][p][c][col] = Wv[c*P+p, db*SB+col]
    Wv_h = np.ascontiguousarray(
        np.asarray(Wv, np.float32).reshape(DC, P, 2, SB)
        .transpose(2, 1, 0, 3).astype(BF16NP)
    )
    bq_s = np.asarray(bq, np.float32) * NORM
    bq_h = np.ascontiguousarray(bq_s.reshape(DC, P).T)
    bk_h = np.ascontiguousarray(np.asarray(bk, np.float32).reshape(DC, P).T)
    bo_h = np.ascontiguousarray(np.asarray(bo, np.float32).reshape(DC, P).T)
    bv_h = np.ascontiguousarray(np.asarray(bv, np.float32).reshape(1, D))

    in_maps = []
    for c in range(8):
        b, h = c // 2, c % 2
        ssl = slice(h * SC, (h + 1) * SC)
        in_maps.append(
            {
                "xT": np.ascontiguousarray(x[b, ssl, :].T.astype(BF16NP)),
                "yT": np.ascontiguousarray(y[b, ssl, :].T.astype(BF16NP)),
                "adjT": np.ascontiguousarray(adj[b, ssl, :].T.astype(BF16NP)),
                "Wq": Wq_h, "Wk": Wk_h, "Wv": Wv_h, "Wo": Wo_h,
                "bq": bq_h, "bk": bk_h, "bv": bv_h, "bo": bo_h,
            }
        )

    nc = _get_nc()
    res = run_bass_kernel_spmd(nc, in_maps, list(range(8)), trace=_trace)
    if _trace:
        _CACHE["last_exec_time_ns"] = res.exec_time_ns
        _CACHE["last_trace"] = (
            res.instructions_and_trace[1] if res.instructions_and_trace else None
        )

    out = np.empty((4, S, D), np.float32)
    for c in range(8):
        b, h = c // 2, c % 2
        out[b, h * SC : (h + 1) * SC, :] = res.results[c]["outT"].T
    return out
